# revision 1
# baseline (speedup 1.0000x reference)
"""Multi-head self-attention TRN2 Bass kernel.

Reference computation (per batch n):
  q = wq @ x; k = wk @ x; v = wv @ x            (1x1 conv == channel matmul)
  per (t, h): score = (q_th^T k_th) * sqrt(D); attn = softmax(score)
  o = attn @ v;  out = wp @ o + bp

Sharding: data-parallel over N=8 across the 8 NeuronCores (one batch each).

Precision strategy (target ~2e-4 rel err):
  - q,k projections: fp16 hi/lo 3-split matmuls (fp32-quality) since softmax
    logits have std ~64 and amplify q/k errors by ~64x.
  - v projection, attn@v, output projection: single-pass fp16 (errors enter
    the output linearly at ~5e-4).
  - score matmul: plain fp32 (exact); exp/softmax stats in fp32.
  - sqrt(D) score scale folded into wq host-side.
"""
import os
import numpy as np
from contextlib import ExitStack

PHASE = float(os.environ.get("KERNEL_PHASE", "6"))

import concourse.bass as bass
import concourse.tile as tile
from concourse import bacc, mybir
from concourse.bass_utils import run_bass_kernel_spmd

N, C, T, V = 8, 256, 128, 128
OUT, H, D = 512, 8, 64
TV = T * V
TC = 8              # t-values per pipeline chunk
NCHUNK = T // TC    # 16
F32 = mybir.dt.float32
F16 = mybir.dt.float16

_CACHE = {}


def _build(nchunk=NCHUNK, debug=False):
    nc = bacc.Bacc("TRN2", target_bir_lowering=False, debug=debug)
    x_d = nc.dram_tensor("x", (C, TV), F32, kind="ExternalInput")
    wqh_d = nc.dram_tensor("wqh", (C, OUT), F16, kind="ExternalInput")
    wql_d = nc.dram_tensor("wql", (C, OUT), F16, kind="ExternalInput")
    wkh_d = nc.dram_tensor("wkh", (C, OUT), F16, kind="ExternalInput")
    wkl_d = nc.dram_tensor("wkl", (C, OUT), F16, kind="ExternalInput")
    wvt_d = nc.dram_tensor("wvt", (C, OUT), F16, kind="ExternalInput")
    wpt_d = nc.dram_tensor("wpt", (OUT, OUT), F16, kind="ExternalInput")
    bp_d = nc.dram_tensor("bpr", (128, 4), F32, kind="ExternalInput")
    out_d = nc.dram_tensor("out", (OUT, TV), F32, kind="ExternalOutput")

    with ExitStack() as ctx:
        tc = ctx.enter_context(tile.TileContext(nc))
        singles = ctx.enter_context(tc.tile_pool(name="singles", bufs=1))
        xpool = ctx.enter_context(tc.tile_pool(name="xp", bufs=2))
        xsplit = ctx.enter_context(tc.tile_pool(name="xs", bufs=2))
        qkpool = ctx.enter_context(tc.tile_pool(name="qk", bufs=2))
        vpool = ctx.enter_context(tc.tile_pool(name="vp", bufs=2))
        atpool = ctx.enter_context(tc.tile_pool(name="at", bufs=8))
        stats = ctx.enter_context(tc.tile_pool(name="st", bufs=8))
        opool = ctx.enter_context(tc.tile_pool(name="op", bufs=2))
        outpool = ctx.enter_context(tc.tile_pool(name="outp", bufs=2))
        projps = ctx.enter_context(tc.tile_pool(name="pps", bufs=2, space="PSUM"))
        scoreps = ctx.enter_context(tc.tile_pool(name="sps", bufs=3, space="PSUM"))
        outps = ctx.enter_context(tc.tile_pool(name="ops2", bufs=1, space="PSUM"))
        ops_ps = ctx.enter_context(tc.tile_pool(name="ops", bufs=2, space="PSUM"))

        FC = TC * V  # free size per chunk (1024)

        def load_split(cc):
            # x chunk load + fp16 hi/lo split
            tv0 = cc * FC
            x_t = xpool.tile([128, 2, FC], F32, tag="x", name="x_t")
            nc.sync.dma_start(
                x_t[:],
                x_d[:].rearrange("(a p) f -> p a f", a=2)[:, :, tv0:tv0 + FC])
            xh = xsplit.tile([128, 2, FC], F16, tag="xh", name="xh")
            xl = xsplit.tile([128, 2, FC], F16, tag="xl", name="xl")
            nc.vector.tensor_copy(xh[:], x_t[:])
            nc.vector.tensor_tensor(out=xl[:], in0=x_t[:], in1=xh[:],
                                    op=mybir.AluOpType.subtract)
            return xh, xl

        splits = {0: load_split(0)}

        # --- weights to SBUF (once) ---
        wq_sb = [singles.tile([128, 2, OUT], F16, tag=f"wq{i}", name=f"wq{i}")
                 for i in range(2)]
        wk_sb = [singles.tile([128, 2, OUT], F16, tag=f"wk{i}", name=f"wk{i}")
                 for i in range(2)]
        nc.sync.dma_start(wq_sb[0][:], wqh_d[:].rearrange("(a p) f -> p a f", a=2))
        nc.sync.dma_start(wq_sb[1][:], wql_d[:].rearrange("(a p) f -> p a f", a=2))
        nc.sync.dma_start(wk_sb[0][:], wkh_d[:].rearrange("(a p) f -> p a f", a=2))
        nc.sync.dma_start(wk_sb[1][:], wkl_d[:].rearrange("(a p) f -> p a f", a=2))
        wvt_sb = singles.tile([128, 2, OUT], F16, tag="wvt")
        nc.sync.dma_start(wvt_sb[:], wvt_d[:].rearrange("(a p) f -> p a f", a=2))
        wpt_sb = singles.tile([128, 4, OUT], F16, tag="wpt")
        nc.sync.dma_start(wpt_sb[:], wpt_d[:].rearrange("(a p) f -> p a f", a=4))
        bp_sb = singles.tile([128, 4], F32, tag="bp")
        nc.sync.dma_start(bp_sb[:], bp_d[:])

        for cc in range(nchunk):
            tv0 = cc * FC
            if cc + 1 < nchunk:
                splits[cc + 1] = load_split(cc + 1)
            xh, xl = splits.pop(cc)

            # --- q, k projections (3-split fp16) ---
            q_sb = qkpool.tile([128, 4, FC], F32, tag="q")
            k_sb = qkpool.tile([128, 4, FC], F32, tag="k")
            for dst, w_sb in ((q_sb, wq_sb), (k_sb, wk_sb)):
                for ot in range(4):
                    pss = [projps.tile([128, 512], F32, tag="pps", name=f"pps{nb}")
                           for nb in range(2)]
                    combos = [(xh, w_sb[0]), (xh, w_sb[1]), (xl, w_sb[0])]
                    for ci, (xa, wb) in enumerate(combos):
                        for ct in range(2):
                            for nb in range(2):
                                nc.tensor.matmul(
                                    pss[nb][:],
                                    wb[:, ct, ot * 128:(ot + 1) * 128],
                                    xa[:, ct, nb * 512:(nb + 1) * 512],
                                    start=(ci == 0 and ct == 0),
                                    stop=(ci == 2 and ct == 1))
                    for nb in range(2):
                        if ot % 2 == 0:
                            nc.scalar.copy(dst[:, ot, nb * 512:(nb + 1) * 512], pss[nb][:])
                        else:
                            nc.vector.tensor_copy(dst[:, ot, nb * 512:(nb + 1) * 512], pss[nb][:])

            if PHASE <= 1:
                for ot in range(4):
                    nc.sync.dma_start(out_d[ot * 128:(ot + 1) * 128, tv0:tv0 + FC],
                                      q_sb[:, ot, :])
                continue

            # --- v projection, transposed layout: vT[t] = [tokens(128), OUT] ---
            vT = vpool.tile([128, TC, OUT], F16, tag="vT")
            for tl in range(TC):
                ps = projps.tile([128, 512], F32, tag="pps")
                for ct in range(2):
                    nc.tensor.matmul(ps[:],
                                     xh[:, ct, tl * 128:(tl + 1) * 128],
                                     wvt_sb[:, ct, :],
                                     start=(ct == 0), stop=(ct == 1))
                nc.scalar.copy(vT[:, tl, :], ps[:])

            if PHASE <= 2:
                vf = outpool.tile([128, TC, OUT], F32, tag="vf", name="vf")
                nc.vector.tensor_copy(vf[:], vT[:])
                nc.sync.dma_start(out_d[0:128, tv0:tv0 + 4096], vf[:].rearrange("p a b -> p (a b)"))
                continue

            # --- attention ---
            for tl in range(TC):
                o_ps = ops_ps.tile([128, 4, 128], F32, tag="ops", name="o_ps") \
                    if PHASE > 4 else None
                for bb in range(2):  # 4-instance batches over heads
                    sps = scoreps.tile([128, 4, 128], F32, tag="sps")
                    negmax = stats.tile([128, 4], F32, tag="negmax")
                    rowsum = stats.tile([128, 4], F32, tag="rowsum")
                    recip = stats.tile([128, 4], F32, tag="recip")
                    for s in range(4):
                        h = 2 * s + bb  # one operand base_partition per bank
                        ot, po = h // 2, (h % 2) * 64
                        nc.tensor.matmul(
                            sps[:, s, :],
                            q_sb[po:po + 64, ot, tl * 128:(tl + 1) * 128],
                            k_sb[po:po + 64, ot, tl * 128:(tl + 1) * 128],
                            start=True, stop=True)
                    if PHASE == 2.1:
                        af = outpool.tile([128, 4, 128], F32, tag="af", name="af")
                        nc.vector.tensor_copy(af[:], sps[:])
                        nc.sync.dma_start(
                            out_d[0:128, (tl * 2 + bb) * 512:(tl * 2 + bb + 1) * 512],
                            af[:].rearrange("p a b -> p (a b)"))
                        continue
                    nc.vector.tensor_reduce(negmax[:], sps[:],
                                            axis=mybir.AxisListType.X,
                                            op=mybir.AluOpType.max, negate=True)
                    if PHASE == 2.2:
                        af = outpool.tile([128, 4], F32, tag="af22", name="af")
                        nc.vector.tensor_copy(af[:], negmax[:])
                        nc.sync.dma_start(
                            out_d[0:128, (tl * 2 + bb) * 4:(tl * 2 + bb + 1) * 4],
                            af[:])
                        continue
                    exp_t = atpool.tile([128, 4, 128], F16, tag="exp")
                    for s in range(4):
                        nc.scalar.activation(exp_t[:, s, :], sps[:, s, :],
                                             mybir.ActivationFunctionType.Exp,
                                             bias=negmax[:, s:s + 1])
                    nc.vector.tensor_reduce(rowsum[:], exp_t[:],
                                            axis=mybir.AxisListType.X,
                                            op=mybir.AluOpType.add)
                    if PHASE == 2.3:
                        af = outpool.tile([128, 4, 128], F32, tag="af", name="af")
                        nc.vector.tensor_copy(af[:], exp_t[:])
                        nc.sync.dma_start(
                            out_d[0:128, (tl * 2 + bb) * 512:(tl * 2 + bb + 1) * 512],
                            af[:].rearrange("p a b -> p (a b)"))
                        continue
                    nc.vector.reciprocal(recip[:], rowsum[:])
                    attn_t = atpool.tile([128, 4, 128], F16, tag="attn")
                    nc.vector.tensor_tensor(
                        out=attn_t[:], in0=exp_t[:],
                        in1=recip[:].unsqueeze(2).broadcast_to([128, 4, 128]),
                        op=mybir.AluOpType.mult)
                    if PHASE <= 3:
                        af = outpool.tile([128, 4, 128], F32, tag="af", name="af")
                        nc.vector.tensor_copy(af[:], attn_t[:])
                        nc.sync.dma_start(
                            out_d[0:128, (tl * 2 + bb) * 512:(tl * 2 + bb + 1) * 512],
                            af[:].rearrange("p a b -> p (a b)"))
                        continue
                    attnT = atpool.tile([128, 4, 128], F16, tag="attnT")
                    nc.sync.dma_start_transpose(attnT[:], attn_t[:])
                    if PHASE <= 4:
                        af = outpool.tile([128, 4, 128], F32, tag="af", name="af")
                        nc.vector.tensor_copy(af[:], attnT[:])
                        nc.sync.dma_start(
                            out_d[0:128, (tl * 2 + bb) * 512:(tl * 2 + bb + 1) * 512],
                            af[:].rearrange("p a b -> p (a b)"))
                        continue
                    for s in range(4):
                        h = 2 * s + bb
                        ot, po = h // 2, (h % 2) * 64
                        nc.tensor.matmul(
                            o_ps[po:po + 64, ot, :],
                            vT[:, tl, h * 64:(h + 1) * 64],
                            attnT[:, s, :],
                            start=True, stop=True)
                if PHASE <= 4:
                    continue
                # stage o (natural layout) for the output projection
                g, tg = tl // 4, tl % 4
                if tg == 0:
                    o_g = opool.tile([128, 4, 4, 128], F16, tag="og")
                nc.scalar.copy(o_g[:, :, tg, :], o_ps[:])
                if PHASE <= 5 and tg == 3:
                    of = outpool.tile([128, 4, 4, 128], F32, tag="of", name="of")
                    nc.vector.tensor_copy(of[:], o_g[:])
                    nc.sync.dma_start(
                        out_d[0:128, tv0 + g * 2048:tv0 + (g + 1) * 2048],
                        of[:].rearrange("p a b c -> p (a b c)"))
                    continue
                if tg == 3:
                    outsb = outpool.tile([128, 4, 512], F32, tag="out", name="outsb")
                    for mt in range(4):
                        ps = outps.tile([128, 512], F32, tag="outps", name="outps")
                        for kt in range(4):
                            nc.tensor.matmul(ps[:],
                                             wpt_sb[:, kt, mt * 128:(mt + 1) * 128],
                                             o_g[:, kt, :, :],
                                             start=(kt == 0), stop=(kt == 3))
                        nc.scalar.add(outsb[:, mt, :], ps[:], bp_sb[:, mt:mt + 1])
                    nc.sync.dma_start(
                        out_d[:].rearrange("(a p) f -> p a f", a=4)
                        [:, :, tv0 + g * 512:tv0 + (g + 1) * 512],
                        outsb[:])

    nc.compile()
    return nc


def _prep_weights(wq, wk, wv, wp, bp):
    wqs = (wq * 8.0).T.astype(np.float32)         # fold sqrt(D)=8 into q
    wkt = wk.T.astype(np.float32)
    wqh = wqs.astype(np.float16)
    wql = (wqs - wqh.astype(np.float32)).astype(np.float16)
    wkh = wkt.astype(np.float16)
    wkl = (wkt - wkh.astype(np.float32)).astype(np.float16)
    wvt = wv.T.astype(np.float16)
    wpt = wp.T.astype(np.float16)
    bpr = np.ascontiguousarray(bp.astype(np.float32).reshape(4, 128).T)
    return dict(wqh=wqh, wql=wql, wkh=wkh, wkl=wkl, wvt=wvt, wpt=wpt, bpr=bpr)


def kernel(x, wq, wk, wv, wp, bp):
    x = np.asarray(x, dtype=np.float32)
    w = _prep_weights(np.asarray(wq), np.asarray(wk), np.asarray(wv),
                      np.asarray(wp), np.asarray(bp))
    if "nc" not in _CACHE:
        _CACHE["nc"] = _build()
    nc = _CACHE["nc"]
    in_maps = []
    for n in range(N):
        m = dict(w)
        m["x"] = np.ascontiguousarray(x[n].reshape(C, TV))
        in_maps.append(m)
    res = run_bass_kernel_spmd(nc, in_maps, core_ids=list(range(N)))
    out = np.stack([r["out"].reshape(OUT, T, V) for r in res.results])
    return out

